# revision 1
# baseline (speedup 1.0000x reference)
"""BlendShapes model kernel for 8 Trainium2 NeuronCores.

Computation (reference):
    pose_repr = pose[:, 1:].reshape(B, 23, 9) - eye      # (B, J, 9)
    per-joint MLP 9 -> 18 -> 32 -> 8 (ReLU between)      # coff (B, J, 8)
    basis_full = basis[:, None] * mask[:, :, None, None]  # (V, J, 8, 3)
    res = einsum('bjk,vjkc->bvc', coff, basis_full)       # (B, V, 3)

Mapping:
  - Vertices are sharded across the 8 cores (V=6890 padded to 8*864=6912).
  - Each core computes the full MLP with activations laid out transposed
    ([features, batch]) so the final coefficients coff^T [J*8, B] feed the
    big matmul's stationary operand directly - no on-chip transposes.
  - All matmul operands are fp16 (1 cycle/row on the PE; fp32/f32r run at 4).
    basis values (~1e-4) would be subnormal in fp16, so the mask carries an
    exact 2^13 scale (bf16) applied in the on-chip basis*mask product; the
    PSUM->SBUF evacuation multiplies by 2^-13 (exact) while converting to f32.
  - Joints are processed in chunks of 4 (3 for the tail) with block-diagonal
    weights packed on the host, so each MLP layer chunk is one PE matmul.
  - The output (B x Vc*3 slice per core) is PSUM-accumulated over K = 184
    (split 128 + 56), evacuated via ACT/DVE, and streamed to HBM per b-tile.
"""

import numpy as np

N_VERT, N_JOINT, BPJ, BATCH = 6890, 23, 8, 1024
VPAD = 6912  # 8 * 864
VC = VPAD // 8  # 864 vertices per core
VC3 = VC * 3  # 2592
# Main matmul N tiling: bank-aligned 512-wide tiles (+ a 32 tail), grouped in
# pairs that share one 2-bank PSUM tile so weights load once per K chunk.
NT_BOUNDS = [0, 512, 1024, 1536, 2048, 2560, 2592]
NT_PAIRS = [(0, 1), (2, 3), (4, 5)]
NB = BATCH // 128  # 8 b-tiles

# Unified joint chunking: the same joint groups for all three MLP layers so
# every matmul's rhs is an entire [K, :] tile (base partition 0).
CHUNKS = [(0, 4), (4, 8), (8, 12), (12, 16), (16, 20), (20, 23)]
NCH = len(CHUNKS)

def _offsets(mpj):
    offs, col = [], 0
    for js, je in CHUNKS:
        offs.append(col)
        col += (je - js) * mpj
    return offs, col

W1_OFF, W1_TOT = _offsets(18)  # 414
W2_OFF, W2_TOT = _offsets(32)  # 736
W3_OFF, W3_TOT = _offsets(8)   # 184
W2_OFF = [W1_TOT + o for o in W2_OFF]
W3_OFF = [W1_TOT + W2_TOT + o for o in W3_OFF]
W_COLS = W1_TOT + W2_TOT + W3_TOT  # 1334

# bias_all columns: [0:6] L1 bias, [6:12] L2 bias, [12:18] L3 bias (all per
# chunk), [18:24] eye vectors per chunk (for pose_repr = pose - eye).
BIAS_COLS = 24
BSCALE = 8192.0  # 2**13, exact in bf16
DESCALE = 1.0 / 8192.0  # exact in f32

_CACHED = {}


def _build_nc():
    import concourse.tile as tile
    from concourse import bacc, mybir
    from contextlib import ExitStack

    dt = mybir.dt
    f32, f16, bf16 = dt.float32, dt.float16, dt.bfloat16
    AF = mybir.ActivationFunctionType
    ALU = mybir.AluOpType

    nc = bacc.Bacc(None, target_bir_lowering=False)

    pose_t = nc.dram_tensor("pose_t", [207, BATCH], f16, kind="ExternalInput")
    basis_t = nc.dram_tensor("basis_t", [BPJ, VC3], f32, kind="ExternalInput")
    mask3 = nc.dram_tensor("mask3", [N_JOINT, VC3], bf16, kind="ExternalInput")
    w_all = nc.dram_tensor("w_all", [128, W_COLS], f16, kind="ExternalInput")
    bias_all = nc.dram_tensor("bias_all", [128, BIAS_COLS], f32, kind="ExternalInput")
    res = nc.dram_tensor("res", [BATCH, VC3], f32, kind="ExternalOutput")

    with ExitStack() as ctx:
        tc = ctx.enter_context(tile.TileContext(nc))
        const = ctx.enter_context(tc.tile_pool(name="const", bufs=1))
        work = ctx.enter_context(tc.tile_pool(name="work", bufs=1))
        outp = ctx.enter_context(tc.tile_pool(name="outp", bufs=2))
        pmlp = ctx.enter_context(tc.tile_pool(name="pmlp", bufs=2, space="PSUM"))
        pmain = ctx.enter_context(tc.tile_pool(name="pmain", bufs=3, space="PSUM"))

        # ---- input DMAs, all on the sync queue in critical-path order:
        # bias (gates eye-sub), weights, pose (gates MLP), then basis/mask
        # (needed only once the main matmul starts, after the MLP).
        bias_sb = const.tile([128, BIAS_COLS], f32, tag="bias")
        nc.sync.dma_start(out=bias_sb[:], in_=bias_all[:, :])
        w_sb = const.tile([128, W_COLS], f16, tag="w")
        nc.sync.dma_start(out=w_sb[:], in_=w_all[:, :])

        pose_c = []
        for c, (js, je) in enumerate(CHUNKS):
            K = 9 * (je - js)
            t = work.tile([K, BATCH], f16, tag=f"pose_{c}", name=f"pose_{c}")
            nc.sync.dma_start(out=t[:], in_=pose_t[9 * js : 9 * js + K, :])
            pose_c.append(t)

        bf_a = work.tile([128, VC3], f32, tag="bf_a")
        bf_b = work.tile([56, VC3], f32, tag="bf_b")
        mk_a = work.tile([128, VC3], bf16, tag="mk_a")
        mk_b = work.tile([56, VC3], bf16, tag="mk_b")
        bfm_a = work.tile([128, VC3], f16, tag="bfm_a")
        bfm_b = work.tile([56, VC3], f16, tag="bfm_b")
        nc.sync.dma_start(out=bf_a[:], in_=basis_t[:, :].partition_broadcast(16))
        nc.sync.dma_start(
            out=mk_a[:], in_=mask3[0:16, :][:, None, :].broadcast_to([16, BPJ, VC3])
        )
        nc.sync.dma_start(out=bf_b[:], in_=basis_t[:, :].partition_broadcast(7))
        nc.sync.dma_start(
            out=mk_b[:], in_=mask3[16:23, :][:, None, :].broadcast_to([7, BPJ, VC3])
        )

        # pose_repr = pose - eye (in place, fp16, DVE 2x mode)
        for c, (js, je) in enumerate(CHUNKS):
            K = 9 * (je - js)
            nc.vector.tensor_scalar(
                out=pose_c[c][:],
                in0=pose_c[c][:],
                scalar1=bias_sb[0:K, 18 + c : 19 + c],
                scalar2=None,
                op0=ALU.subtract,
            )

        # basis_full = (basis * 2^13) * mask on GPSIMD -> fp16 product tiles
        for t in range(len(NT_BOUNDS) - 1):
            sl = slice(NT_BOUNDS[t], NT_BOUNDS[t + 1])
            nc.gpsimd.tensor_tensor(
                out=bfm_a[:, sl], in0=bf_a[:, sl], in1=mk_a[:, sl], op=ALU.mult
            )
            nc.gpsimd.tensor_tensor(
                out=bfm_b[:, sl], in0=bf_b[:, sl], in1=mk_b[:, sl], op=ALU.mult
            )

        coffT_a = work.tile([128, BATCH], f16, tag="coffT_a")
        coffT_b = work.tile([56, BATCH], f16, tag="coffT_b")
        h1 = {}
        h2 = {}
        coff_c = {}

        def mlp_epilogue(use_act, dst, ps, bias_ap, relu):
            # Split between ACT and DVE so the 2-slot PSUM chain advances two
            # tiles per epilogue latency instead of one. ACT's Copy cannot
            # take an AP bias, so bias-only (L3) epilogues go to DVE.
            if use_act:
                nc.scalar.activation(dst, ps, AF.Relu, bias=bias_ap)
            elif relu:
                nc.vector.tensor_scalar(
                    out=dst, in0=ps, scalar1=bias_ap, scalar2=0.0,
                    op0=ALU.add, op1=ALU.max,
                )
            else:
                nc.vector.tensor_scalar(
                    out=dst, in0=ps, scalar1=bias_ap, scalar2=None, op0=ALU.add
                )

        def mlp_half(h):
            hs = slice(h * 512, (h + 1) * 512)
            # L1: 9nj -> 18nj, ReLU(x + b)
            for c, (js, je) in enumerate(CHUNKS):
                nj = je - js
                K, M = 9 * nj, 18 * nj
                off = W1_OFF[c]
                ps = pmlp.tile([M, 512], f32, tag="psmlp", name=f"ps1_{c}_{h}")
                nc.tensor.matmul(
                    ps[:], lhsT=w_sb[0:K, off : off + M], rhs=pose_c[c][:, hs],
                    start=True, stop=True,
                )
                if h == 0:
                    h1[c] = work.tile([M, BATCH], f16, tag=f"h1_{c}", name=f"h1_{c}")
                mlp_epilogue(True, h1[c][:, hs], ps[:], bias_sb[0:M, c : c + 1], True)
            # L2: 18nj -> 32nj, ReLU
            for c, (js, je) in enumerate(CHUNKS):
                nj = je - js
                K, M = 18 * nj, 32 * nj
                off = W2_OFF[c]
                ps = pmlp.tile([M, 512], f32, tag="psmlp", name=f"ps2_{c}_{h}")
                nc.tensor.matmul(
                    ps[:], lhsT=w_sb[0:K, off : off + M], rhs=h1[c][:, hs],
                    start=True, stop=True,
                )
                if h == 0:
                    h2[c] = work.tile([M, BATCH], f16, tag=f"h2_{c}", name=f"h2_{c}")
                mlp_epilogue(c % 2 == 0, h2[c][:, hs], ps[:], bias_sb[0:M, 6 + c : 7 + c], True)
            # L3: 32nj -> 8nj, bias only, into per-chunk coff tiles; small
            # SBUF->SBUF DMAs (gpsimd queue) merge them into coffT_a/coffT_b
            # (DMA is the only engine that can shift partition bases).
            for c, (js, je) in enumerate(CHUNKS):
                nj = je - js
                K, M = 32 * nj, 8 * nj
                off = W3_OFF[c]
                ps = pmlp.tile([M, 512], f32, tag="psmlp", name=f"ps3_{c}_{h}")
                nc.tensor.matmul(
                    ps[:], lhsT=w_sb[0:K, off : off + M], rhs=h2[c][:, hs],
                    start=True, stop=True,
                )
                if h == 0:
                    coff_c[c] = work.tile(
                        [M, BATCH], f16, tag=f"coff_{c}", name=f"coff_{c}"
                    )
                mlp_epilogue(
                    False, coff_c[c][:, hs], ps[:], bias_sb[0:M, 12 + c : 13 + c], False
                )
                if c < 4:
                    dst = coffT_a[32 * c : 32 * c + M, hs]
                else:
                    r0 = 32 * (c - 4)
                    dst = coffT_b[r0 : r0 + M, hs]
                nc.gpsimd.dma_start(out=dst, in_=coff_c[c][:, hs])

        def main_btile(bt):
            bsl = slice(bt * 128, (bt + 1) * 128)
            ostrip = outp.tile([128, VC3], f32, tag="ostrip", name=f"ostrip_{bt}")
            for p, (t0, t1) in enumerate(NT_PAIRS):
                s0 = slice(NT_BOUNDS[t0], NT_BOUNDS[t0 + 1])
                s1 = slice(NT_BOUNDS[t1], NT_BOUNDS[t1 + 1])
                n0 = NT_BOUNDS[t0 + 1] - NT_BOUNDS[t0]
                n1 = NT_BOUNDS[t1 + 1] - NT_BOUNDS[t1]
                ps = pmain.tile([128, 1024], f32, tag="ps", name=f"ps_{bt}_{p}")
                # K chunk a for both tiles of the pair (weights loaded once),
                # then K chunk b accumulating on top. Tile 1 starts at column
                # 512 so each matmul output stays within one PSUM bank.
                nc.tensor.matmul(
                    ps[:, 0:n0], lhsT=coffT_a[:, bsl], rhs=bfm_a[:, s0],
                    start=True, stop=False,
                )
                nc.tensor.matmul(
                    ps[:, 512 : 512 + n1], lhsT=coffT_a[:, bsl], rhs=bfm_a[:, s1],
                    start=True, stop=False,
                )
                nc.tensor.matmul(
                    ps[:, 0:n0], lhsT=coffT_b[:, bsl], rhs=bfm_b[:, s0],
                    start=False, stop=True,
                )
                nc.tensor.matmul(
                    ps[:, 512 : 512 + n1], lhsT=coffT_b[:, bsl], rhs=bfm_b[:, s1],
                    start=False, stop=True,
                )
                # evacuate PSUM -> SBUF with the exact 2^-13 descale; the two
                # tiles are contiguous in PSUM (columns 0..512+n1) only when
                # n0 == 512, which holds for every pair by construction.
                osl = slice(NT_BOUNDS[t0], NT_BOUNDS[t0] + 512 + n1)
                if (bt * len(NT_PAIRS) + p) % 2 == 0:
                    nc.scalar.activation(
                        ostrip[:, osl], ps[:, 0 : 512 + n1], AF.Copy, scale=DESCALE
                    )
                else:
                    nc.vector.tensor_scalar(
                        out=ostrip[:, osl], in0=ps[:, 0 : 512 + n1], scalar1=DESCALE,
                        scalar2=None, op0=ALU.mult,
                    )
            nc.sync.dma_start(out=res[bsl, :], in_=ostrip[:])

        # First batch-half of the MLP, then its 4 output b-tiles (overlapping
        # the second half's MLP epilogues), then the rest.
        mlp_half(0)
        for bt in range(4):
            main_btile(bt)
        mlp_half(1)
        for bt in range(4, NB):
            main_btile(bt)

    nc.finalize()
    return nc


def _pack_host(pose, basis, mask, w1, b1, w2, b2, w3, b3):
    import ml_dtypes

    pose_t = np.ascontiguousarray(
        pose[:, 1:].reshape(BATCH, 207).T.astype(np.float16)
    )  # [207, B] rows are (j, i)

    basis_t = np.zeros((BPJ, VPAD * 3), np.float32)  # [k, (v, c)]
    basis_t[:, : N_VERT * 3] = basis.transpose(1, 0, 2).reshape(BPJ, N_VERT * 3)

    mask3 = np.zeros((N_JOINT, VPAD * 3), ml_dtypes.bfloat16)  # [j, (v, c)]
    mask3[:, : N_VERT * 3] = (np.repeat(mask.T, 3, axis=1) * BSCALE).astype(
        ml_dtypes.bfloat16
    )

    w_all = np.zeros((128, W_COLS), np.float16)
    bias_all = np.zeros((128, BIAS_COLS), np.float32)
    for (js, je), o1, o2, o3 in zip(CHUNKS, W1_OFF, W2_OFF, W3_OFF):
        for t, j in enumerate(range(js, je)):
            w_all[t * 9 : (t + 1) * 9, o1 + t * 18 : o1 + (t + 1) * 18] = w1[j]
            w_all[t * 18 : (t + 1) * 18, o2 + t * 32 : o2 + (t + 1) * 32] = w2[j]
            w_all[t * 32 : (t + 1) * 32, o3 + t * 8 : o3 + (t + 1) * 8] = w3[j]
    for c, (js, je) in enumerate(CHUNKS):
        nj = je - js
        bias_all[0 : 18 * nj, c] = b1[js:je].reshape(-1)
        bias_all[0 : 32 * nj, 6 + c] = b2[js:je].reshape(-1)
        bias_all[0 : 8 * nj, 12 + c] = b3[js:je].reshape(-1)
        # eye vector for this chunk's pose rows: 1.0 at i in {0, 4, 8}
        ev = np.zeros((nj, 9), np.float32)
        ev[:, [0, 4, 8]] = 1.0
        bias_all[0 : 9 * nj, 18 + c] = ev.reshape(-1)

    return pose_t, basis_t, mask3, w_all, bias_all


def _in_maps(pose, basis, mask, w1, b1, w2, b2, w3, b3):
    pose_t, basis_t, mask3, w_all, bias_all = _pack_host(
        np.asarray(pose, np.float32),
        np.asarray(basis, np.float32),
        np.asarray(mask, np.float32),
        np.asarray(w1, np.float32),
        np.asarray(b1, np.float32),
        np.asarray(w2, np.float32),
        np.asarray(b2, np.float32),
        np.asarray(w3, np.float32),
        np.asarray(b3, np.float32),
    )
    maps = []
    for i in range(8):
        c0 = i * VC3
        maps.append(
            {
                "pose_t": pose_t,
                "basis_t": np.ascontiguousarray(basis_t[:, c0 : c0 + VC3]),
                "mask3": np.ascontiguousarray(mask3[:, c0 : c0 + VC3]),
                "w_all": w_all,
                "bias_all": bias_all,
            }
        )
    return maps


def kernel(pose, basis, mask, w1, b1, w2, b2, w3, b3):
    from concourse.bass_utils import run_bass_kernel_spmd

    if "nc" not in _CACHED:
        _CACHED["nc"] = _build_nc()
    nc = _CACHED["nc"]

    maps = _in_maps(pose, basis, mask, w1, b1, w2, b2, w3, b3)
    r = run_bass_kernel_spmd(nc, maps, core_ids=list(range(8)))
    out = np.concatenate(
        [m["res"].reshape(BATCH, VC, 3) for m in r.results], axis=1
    )
    return np.ascontiguousarray(out[:, :N_VERT, :])



# revision 3
# speedup vs baseline: 1.2707x; 1.2707x over previous
"""BlendShapes model kernel for 8 Trainium2 NeuronCores.

Computation (reference):
    pose_repr = pose[:, 1:].reshape(B, 23, 9) - eye      # (B, J, 9)
    per-joint MLP 9 -> 18 -> 32 -> 8 (ReLU between)      # coff (B, J, 8)
    basis_full = basis[:, None] * mask[:, :, None, None]  # (V, J, 8, 3)
    res = einsum('bjk,vjkc->bvc', coff, basis_full)       # (B, V, 3)

Mapping:
  - Vertices sharded across 8 cores (V=6890 padded to 8*864=6912).
  - basis*mask (both pure inputs) is folded on the host into one fp16
    tensor bfm [184, Vc*3] with an exact 2^13 scale (fp16-subnormal guard);
    the PSUM evacuation multiplies by 2^-13 while converting to f32.
  - Each core runs the full MLP with activations transposed ([feat, batch])
    so coff^T feeds the main matmul's stationary operand directly.
  - Joints packed block-diagonally in chunks of 4 (tail 3); both 512-wide
    batch halves of a chunk go into one [M, 1024] PSUM tile -> one epilogue.
  - L3 chunk outputs land at 32-aligned partition offsets of a single PSUM
    tile, so coffT_a/[b] are produced by single whole-tile epilogues.
  - Warm-up matmuls on a zero tile keep the PE active during the input DMA
    phase so the clock ramp (PE_HAM) reaches 2.4 GHz before the MLP.
  - Main matmul: per 128-batch tile, K=184 split 128+56, N tiled in pairs
    sharing a 2-bank PSUM tile; evacuations alternate ACT/DVE; output
    stores alternate the two HWDGE rings (sync / scalar).
"""

import numpy as np

N_VERT, N_JOINT, BPJ, BATCH = 6890, 23, 8, 1024
VPAD = 6912  # 8 * 864
VC = VPAD // 8  # 864 vertices per core
VC3 = VC * 3  # 2592
KTOT = N_JOINT * BPJ  # 184
NT_BOUNDS = [0, 512, 1024, 1536, 2048, 2560, 2592]
NT_PAIRS = [(0, 1), (2, 3), (4, 5)]
NB = BATCH // 128  # 8 b-tiles

CHUNKS = [(0, 4), (4, 8), (8, 12), (12, 16), (16, 20), (20, 23)]


def _offsets(mpj):
    offs, col = [], 0
    for js, je in CHUNKS:
        offs.append(col)
        col += (je - js) * mpj
    return offs, col


W1_OFF, W1_TOT = _offsets(18)  # 414
W2_OFF, W2_TOT = _offsets(32)  # 736
W3_OFF, W3_TOT = _offsets(8)   # 184
W2_OFF = [W1_TOT + o for o in W2_OFF]
W3_OFF = [W1_TOT + W2_TOT + o for o in W3_OFF]
W_COLS = W1_TOT + W2_TOT + W3_TOT  # 1334

# bias_all columns: [0:6] L1 bias, [6:12] L2 bias, [12] L3 bias rows for
# coffT_a (joints 0-15, j-major), [13] same for coffT_b (joints 16-22),
# [14:20] eye vectors per chunk (for pose_repr = pose - eye).
BIAS_COLS = 20
BSCALE = 8192.0  # 2**13, exact in fp16 range after product
DESCALE = 1.0 / 8192.0  # exact in f32
NWARM = 16

_CACHED = {}


def _build_nc():
    import concourse.tile as tile
    from concourse import bacc, mybir
    from contextlib import ExitStack

    dt = mybir.dt
    f32, f16 = dt.float32, dt.float16
    AF = mybir.ActivationFunctionType
    ALU = mybir.AluOpType

    nc = bacc.Bacc(None, target_bir_lowering=False)

    pose_t = nc.dram_tensor("pose_t", [207, BATCH], f16, kind="ExternalInput")
    bfm_t = nc.dram_tensor("bfm_t", [KTOT, VC3], f16, kind="ExternalInput")
    w_all = nc.dram_tensor("w_all", [128, W_COLS], f16, kind="ExternalInput")
    bias_all = nc.dram_tensor("bias_all", [128, BIAS_COLS], f32, kind="ExternalInput")
    res = nc.dram_tensor("res", [BATCH, VC3], f32, kind="ExternalOutput")

    with ExitStack() as ctx:
        tc = ctx.enter_context(tile.TileContext(nc))
        const = ctx.enter_context(tc.tile_pool(name="const", bufs=1))
        work = ctx.enter_context(tc.tile_pool(name="work", bufs=1))
        outp = ctx.enter_context(tc.tile_pool(name="outp", bufs=4))

        # ---- input DMAs. sync ring: MLP critical path; scalar ring: bfm.
        bias_sb = const.tile([128, BIAS_COLS], f32, tag="bias")
        nc.sync.dma_start(out=bias_sb[:], in_=bias_all[:, :])
        w_sb = const.tile([128, W_COLS], f16, tag="w")
        nc.sync.dma_start(out=w_sb[:], in_=w_all[:, :])
        pose_c = []
        for c, (js, je) in enumerate(CHUNKS):
            K = 9 * (je - js)
            t = work.tile([K, BATCH], f16, tag=f"pose_{c}", name=f"pose_{c}")
            nc.sync.dma_start(out=t[:], in_=pose_t[9 * js : 9 * js + K, :])
            pose_c.append(t)

        bfm_a = work.tile([96, VC3], f16, tag="bfm_a")
        bfm_b = work.tile([88, VC3], f16, tag="bfm_b")
        nc.scalar.dma_start(out=bfm_a[:], in_=bfm_t[0:96, :])
        nc.scalar.dma_start(out=bfm_b[:], in_=bfm_t[96:KTOT, :])

        # ---- PE warm-up on a zeroed tile (keeps the clock ramp going while
        # the input DMAs stream; results are never read).
        warm = work.tile([128, 512], f16, tag="warm")
        nc.gpsimd.memset(warm[:], 0)
        with tc.tile_pool(name="pwarm", bufs=1, space="PSUM") as pwarm:
            ps_w = pwarm.tile([128, 512], f32, tag="warm")
            for i in range(NWARM):
                nc.tensor.matmul(
                    ps_w[:], lhsT=warm[:, 0:128], rhs=warm[:],
                    start=True, stop=True,
                )

        # pose_repr = pose - eye (in place, fp16, DVE 2x mode)
        for c, (js, je) in enumerate(CHUNKS):
            K = 9 * (je - js)
            nc.vector.tensor_scalar(
                out=pose_c[c][:],
                in0=pose_c[c][:],
                scalar1=bias_sb[0:K, 14 + c : 15 + c],
                scalar2=None,
                op0=ALU.subtract,
            )

        coffT_a = work.tile([96, BATCH], f16, tag="coffT_a")
        coffT_b = work.tile([88, BATCH], f16, tag="coffT_b")
        h1 = {}
        h2 = {}
        ep_ctr = [0]

        def epilogue(dst, ps, bias_ap, relu):
            # Alternate ACT / DVE so consecutive epilogues overlap.
            use_act = ep_ctr[0] % 2 == 0
            ep_ctr[0] += 1
            if relu and use_act:
                nc.scalar.activation(dst, ps, AF.Relu, bias=bias_ap)
            elif relu:
                nc.vector.tensor_scalar(
                    out=dst, in0=ps, scalar1=bias_ap, scalar2=0.0,
                    op0=ALU.add, op1=ALU.max,
                )
            else:
                nc.vector.tensor_scalar(
                    out=dst, in0=ps, scalar1=bias_ap, scalar2=None, op0=ALU.add
                )

        # ---- MLP, both batch halves per chunk into one [M, 1024] PSUM tile.
        with tc.tile_pool(name="pmlp", bufs=3, space="PSUM") as pmlp:
            # L1: 9nj -> 18nj, ReLU(x + b)
            for c, (js, je) in enumerate(CHUNKS):
                nj = je - js
                K, M = 9 * nj, 18 * nj
                off = W1_OFF[c]
                ps = pmlp.tile([M, 1024], f32, tag="psmlp", name=f"ps1_{c}")
                for h in (0, 1):
                    hs = slice(h * 512, (h + 1) * 512)
                    nc.tensor.matmul(
                        ps[:, hs], lhsT=w_sb[0:K, off : off + M],
                        rhs=pose_c[c][:, hs], start=True, stop=True,
                    )
                h1[c] = work.tile([M, BATCH], f16, tag=f"h1_{c}", name=f"h1_{c}")
                epilogue(h1[c][:], ps[:], bias_sb[0:M, c : c + 1], True)
            # L2: 18nj -> 32nj, ReLU
            for c, (js, je) in enumerate(CHUNKS):
                nj = je - js
                K, M = 18 * nj, 32 * nj
                off = W2_OFF[c]
                ps = pmlp.tile([M, 1024], f32, tag="psmlp", name=f"ps2_{c}")
                for h in (0, 1):
                    hs = slice(h * 512, (h + 1) * 512)
                    nc.tensor.matmul(
                        ps[:, hs], lhsT=w_sb[0:K, off : off + M],
                        rhs=h1[c][:, hs], start=True, stop=True,
                    )
                h2[c] = work.tile([M, BATCH], f16, tag=f"h2_{c}", name=f"h2_{c}")
                epilogue(h2[c][:], ps[:], bias_sb[0:M, 6 + c : 7 + c], True)
            # L3: 32nj -> 8nj, bias only; chunk outputs stack at 32-aligned
            # partition offsets of one PSUM tile -> single whole-tile
            # epilogues produce coffT_a / coffT_b directly.
            ps3a = pmlp.tile([96, 1024], f32, tag="psmlp", name="ps3a")
            ps3b = pmlp.tile([88, 1024], f32, tag="psmlp", name="ps3b")
            for c, (js, je) in enumerate(CHUNKS):
                nj = je - js
                K, M = 32 * nj, 8 * nj
                off = W3_OFF[c]
                if c < 3:
                    dst = ps3a[32 * c : 32 * c + M, :]
                else:
                    r0 = 32 * (c - 3)
                    dst = ps3b[r0 : r0 + M, :]
                for h in (0, 1):
                    hs = slice(h * 512, (h + 1) * 512)
                    nc.tensor.matmul(
                        dst[:, hs], lhsT=w_sb[0:K, off : off + M],
                        rhs=h2[c][:, hs], start=True, stop=True,
                    )
            epilogue(coffT_a[:], ps3a[:], bias_sb[0:96, 12:13], False)
            epilogue(coffT_b[:], ps3b[:], bias_sb[0:88, 13:14], False)

        # ---- main matmul: out[128b, VC3] += coffT.T @ bfm, K = 128 + 56.
        with tc.tile_pool(name="pmain", bufs=4, space="PSUM") as pmain:
            for bt in range(NB):
                bsl = slice(bt * 128, (bt + 1) * 128)
                ostrip = outp.tile([128, VC3], f32, tag="ostrip", name=f"o_{bt}")
                pstiles = []
                for p, (t0, t1) in enumerate(NT_PAIRS):
                    ps = pmain.tile([128, 1024], f32, tag="ps", name=f"ps_{bt}_{p}")
                    pstiles.append(ps)
                # K chunk a across all pairs (stationary loaded once per
                # chunk), then chunk b accumulating on top.
                for ki, (cof, kk) in enumerate(((coffT_a, 96), (coffT_b, 88))):
                    for p, (t0, t1) in enumerate(NT_PAIRS):
                        s0 = slice(NT_BOUNDS[t0], NT_BOUNDS[t0 + 1])
                        s1 = slice(NT_BOUNDS[t1], NT_BOUNDS[t1 + 1])
                        n1 = NT_BOUNDS[t1 + 1] - NT_BOUNDS[t1]
                        ps = pstiles[p]
                        nc.tensor.matmul(
                            ps[:, 0:512], lhsT=cof[:, bsl], rhs=bfm_a[0:kk, s0] if ki == 0 else bfm_b[0:kk, s0],
                            start=ki == 0, stop=ki == 1,
                        )
                        nc.tensor.matmul(
                            ps[:, 512 : 512 + n1], lhsT=cof[:, bsl],
                            rhs=bfm_a[0:kk, s1] if ki == 0 else bfm_b[0:kk, s1],
                            start=ki == 0, stop=ki == 1,
                        )
                # evacuate with the exact 2^-13 descale, alternating ACT/DVE
                for p, (t0, t1) in enumerate(NT_PAIRS):
                    n1 = NT_BOUNDS[t1 + 1] - NT_BOUNDS[t1]
                    osl = slice(NT_BOUNDS[t0], NT_BOUNDS[t0] + 512 + n1)
                    ps = pstiles[p]
                    if ep_ctr[0] % 2 == 0:
                        nc.scalar.activation(
                            ostrip[:, osl], ps[:, 0 : 512 + n1], AF.Copy,
                            scale=DESCALE,
                        )
                    else:
                        nc.vector.tensor_scalar(
                            out=ostrip[:, osl], in0=ps[:, 0 : 512 + n1],
                            scalar1=DESCALE, scalar2=None, op0=ALU.mult,
                        )
                    ep_ctr[0] += 1
                # stores alternate the two HWDGE rings
                if bt % 2 == 0:
                    nc.sync.dma_start(out=res[bsl, :], in_=ostrip[:])
                else:
                    nc.scalar.dma_start(out=res[bsl, :], in_=ostrip[:])

    nc.finalize()
    return nc


def _pack_host(pose, basis, mask, w1, b1, w2, b2, w3, b3):
    pose_t = np.ascontiguousarray(
        pose[:, 1:].reshape(BATCH, 207).T.astype(np.float16)
    )  # [207, B] rows are (j, i)

    # bfm[j*8+k, v*3+c] = basis[v, k, c] * mask[v, j] * 2^13, fp16
    prod = (
        basis[None, :, :, :] * mask.T[:, :, None, None] * BSCALE
    )  # (J, V, 8, 3) f32
    bfm = np.zeros((KTOT, VPAD * 3), np.float16)
    bfm[:, : N_VERT * 3] = prod.transpose(0, 2, 1, 3).reshape(KTOT, N_VERT * 3)

    w_pack = np.zeros((128, W_COLS), np.float16)
    bias_all = np.zeros((128, BIAS_COLS), np.float32)
    for (js, je), o1, o2, o3 in zip(CHUNKS, W1_OFF, W2_OFF, W3_OFF):
        for t, j in enumerate(range(js, je)):
            w_pack[t * 9 : (t + 1) * 9, o1 + t * 18 : o1 + (t + 1) * 18] = w1[j]
            w_pack[t * 18 : (t + 1) * 18, o2 + t * 32 : o2 + (t + 1) * 32] = w2[j]
            w_pack[t * 32 : (t + 1) * 32, o3 + t * 8 : o3 + (t + 1) * 8] = w3[j]
    for c, (js, je) in enumerate(CHUNKS):
        nj = je - js
        bias_all[0 : 18 * nj, c] = b1[js:je].reshape(-1)
        bias_all[0 : 32 * nj, 6 + c] = b2[js:je].reshape(-1)
        ev = np.zeros((nj, 9), np.float32)
        ev[:, [0, 4, 8]] = 1.0
        bias_all[0 : 9 * nj, 14 + c] = ev.reshape(-1)
    bias_all[0:96, 12] = b3[0:12].reshape(-1)
    bias_all[0:88, 13] = b3[12:23].reshape(-1)

    return pose_t, bfm, w_pack, bias_all


def _in_maps(pose, basis, mask, w1, b1, w2, b2, w3, b3):
    pose_t, bfm, w_pack, bias_all = _pack_host(
        np.asarray(pose, np.float32),
        np.asarray(basis, np.float32),
        np.asarray(mask, np.float32),
        np.asarray(w1, np.float32),
        np.asarray(b1, np.float32),
        np.asarray(w2, np.float32),
        np.asarray(b2, np.float32),
        np.asarray(w3, np.float32),
        np.asarray(b3, np.float32),
    )
    maps = []
    for i in range(8):
        c0 = i * VC3
        maps.append(
            {
                "pose_t": pose_t,
                "bfm_t": np.ascontiguousarray(bfm[:, c0 : c0 + VC3]),
                "w_all": w_pack,
                "bias_all": bias_all,
            }
        )
    return maps


def kernel(pose, basis, mask, w1, b1, w2, b2, w3, b3):
    from concourse.bass_utils import run_bass_kernel_spmd

    if "nc" not in _CACHED:
        _CACHED["nc"] = _build_nc()
    nc = _CACHED["nc"]

    maps = _in_maps(pose, basis, mask, w1, b1, w2, b2, w3, b3)
    r = run_bass_kernel_spmd(nc, maps, core_ids=list(range(8)))
    out = np.concatenate(
        [m["res"].reshape(BATCH, VC, 3) for m in r.results], axis=1
    )
    return np.ascontiguousarray(out[:, :N_VERT, :])


# revision 4
# speedup vs baseline: 1.2803x; 1.0076x over previous
"""BlendShapes model kernel for 8 Trainium2 NeuronCores.

Computation (reference):
    pose_repr = pose[:, 1:].reshape(B, 23, 9) - eye      # (B, J, 9)
    per-joint MLP 9 -> 18 -> 32 -> 8 (ReLU between)      # coff (B, J, 8)
    basis_full = basis[:, None] * mask[:, :, None, None]  # (V, J, 8, 3)
    res = einsum('bjk,vjkc->bvc', coff, basis_full)       # (B, V, 3)

Mapping:
  - Vertices sharded across 8 cores (V=6890 padded to 8*864=6912).
  - basis*mask (both pure inputs) is folded on the host into one fp16
    tensor bfm [184, Vc*3] with an exact 2^13 scale (fp16-subnormal guard);
    the PSUM evacuation multiplies by 2^-13 while converting to f32.
  - Each core runs the full MLP with activations transposed ([feat, batch])
    so coff^T feeds the main matmul's stationary operand directly.
  - Joints packed block-diagonally in chunks of 4 (tail 3); both 512-wide
    batch halves of a chunk go into one [M, 1024] PSUM tile -> one epilogue.
  - L3 chunk outputs land at 32-aligned partition offsets of a single PSUM
    tile, so coffT_a/[b] are produced by single whole-tile epilogues.
  - Warm-up matmuls on a zero tile keep the PE active during the input DMA
    phase so the clock ramp (PE_HAM) reaches 2.4 GHz before the MLP.
  - Main matmul: per 128-batch tile, K=184 split 128+56, N tiled in pairs
    sharing a 2-bank PSUM tile; evacuations alternate ACT/DVE; output
    stores alternate the two HWDGE rings (sync / scalar).
"""

import numpy as np
import ml_dtypes

BF16 = ml_dtypes.bfloat16

N_VERT, N_JOINT, BPJ, BATCH = 6890, 23, 8, 1024
VPAD = 6912  # 8 * 864
VC = VPAD // 8  # 864 vertices per core
VC3 = VC * 3  # 2592
KTOT = N_JOINT * BPJ  # 184
NT_BOUNDS = [0, 512, 1024, 1536, 2048, 2560, 2592]
NT_PAIRS = [(0, 1), (2, 3), (4, 5)]
NB = BATCH // 128  # 8 b-tiles

CHUNKS = [(0, 4), (4, 8), (8, 12), (12, 16), (16, 20), (20, 23)]


def _offsets(mpj):
    offs, col = [], 0
    for js, je in CHUNKS:
        offs.append(col)
        col += (je - js) * mpj
    return offs, col


W1_OFF, W1_TOT = _offsets(18)  # 414
W2_OFF, W2_TOT = _offsets(32)  # 736
W3_OFF, W3_TOT = _offsets(8)   # 184
W2_OFF = [W1_TOT + o for o in W2_OFF]
W3_OFF = [W1_TOT + W2_TOT + o for o in W3_OFF]
W_COLS = W1_TOT + W2_TOT + W3_TOT  # 1334

# bias_all columns: [0:6] L1 bias, [6:12] L2 bias, [12] L3 bias rows for
# coffT_a (joints 0-15, j-major), [13] same for coffT_b (joints 16-22),
# [14:20] eye vectors per chunk (for pose_repr = pose - eye).
BIAS_COLS = 20
BSCALE = 8192.0  # 2**13, exact in fp16 range after product
DESCALE = 1.0 / 8192.0  # exact in f32
NWARM = 16

_CACHED = {}


def _build_nc():
    import concourse.tile as tile
    from concourse import bacc, mybir
    from contextlib import ExitStack

    dt = mybir.dt
    f32, f16 = dt.float32, dt.bfloat16
    AF = mybir.ActivationFunctionType
    ALU = mybir.AluOpType

    nc = bacc.Bacc(None, target_bir_lowering=False)

    pose_t = nc.dram_tensor("pose_t", [207, BATCH], f16, kind="ExternalInput")
    bfm_t = nc.dram_tensor("bfm_t", [KTOT, VC3], f16, kind="ExternalInput")
    w_all = nc.dram_tensor("w_all", [128, W_COLS], f16, kind="ExternalInput")
    bias_all = nc.dram_tensor("bias_all", [128, BIAS_COLS], f32, kind="ExternalInput")
    res = nc.dram_tensor("res", [BATCH, VC3], f32, kind="ExternalOutput")

    with ExitStack() as ctx:
        tc = ctx.enter_context(tile.TileContext(nc))
        const = ctx.enter_context(tc.tile_pool(name="const", bufs=1))
        work = ctx.enter_context(tc.tile_pool(name="work", bufs=1))
        outp = ctx.enter_context(tc.tile_pool(name="outp", bufs=4))

        # ---- input DMAs. sync ring: MLP critical path; scalar ring: bfm.
        bias_sb = const.tile([128, BIAS_COLS], f32, tag="bias")
        nc.sync.dma_start(out=bias_sb[:], in_=bias_all[:, :])
        w_sb = const.tile([128, W_COLS], f16, tag="w")
        nc.sync.dma_start(out=w_sb[:], in_=w_all[:, :])
        pose_c = []
        for c, (js, je) in enumerate(CHUNKS):
            K = 9 * (je - js)
            t = work.tile([K, BATCH], f16, tag=f"pose_{c}", name=f"pose_{c}")
            nc.sync.dma_start(out=t[:], in_=pose_t[9 * js : 9 * js + K, :])
            pose_c.append(t)

        bfm_a = work.tile([96, VC3], f16, tag="bfm_a")
        bfm_b = work.tile([88, VC3], f16, tag="bfm_b")
        nc.scalar.dma_start(out=bfm_a[:], in_=bfm_t[0:96, :])
        nc.scalar.dma_start(out=bfm_b[:], in_=bfm_t[96:KTOT, :])

        # pose_repr = pose - eye (in place, fp16, DVE 2x mode)
        for c, (js, je) in enumerate(CHUNKS):
            K = 9 * (je - js)
            nc.vector.tensor_scalar(
                out=pose_c[c][:],
                in0=pose_c[c][:],
                scalar1=bias_sb[0:K, 14 + c : 15 + c],
                scalar2=None,
                op0=ALU.subtract,
            )

        coffT_a = work.tile([96, BATCH], f16, tag="coffT_a")
        coffT_b = work.tile([88, BATCH], f16, tag="coffT_b")
        h1 = {}
        h2 = {}
        ep_ctr = [0]

        def epilogue(dst, ps, bias_ap, relu):
            # Alternate ACT / DVE so consecutive epilogues overlap.
            use_act = ep_ctr[0] % 2 == 0
            ep_ctr[0] += 1
            if relu and use_act:
                nc.scalar.activation(dst, ps, AF.Relu, bias=bias_ap)
            elif relu:
                nc.vector.tensor_scalar(
                    out=dst, in0=ps, scalar1=bias_ap, scalar2=0.0,
                    op0=ALU.add, op1=ALU.max,
                )
            else:
                nc.vector.tensor_scalar(
                    out=dst, in0=ps, scalar1=bias_ap, scalar2=None, op0=ALU.add
                )

        # ---- MLP, both batch halves per chunk into one [M, 1024] PSUM tile.
        with tc.tile_pool(name="pmlp", bufs=3, space="PSUM") as pmlp:
            # L1: 9nj -> 18nj, ReLU(x + b)
            for c, (js, je) in enumerate(CHUNKS):
                nj = je - js
                K, M = 9 * nj, 18 * nj
                off = W1_OFF[c]
                ps = pmlp.tile([M, 1024], f32, tag="psmlp", name=f"ps1_{c}")
                for h in (0, 1):
                    hs = slice(h * 512, (h + 1) * 512)
                    nc.tensor.matmul(
                        ps[:, hs], lhsT=w_sb[0:K, off : off + M],
                        rhs=pose_c[c][:, hs], start=True, stop=True,
                    )
                h1[c] = work.tile([M, BATCH], f16, tag=f"h1_{c}", name=f"h1_{c}")
                epilogue(h1[c][:], ps[:], bias_sb[0:M, c : c + 1], True)
            # L2: 18nj -> 32nj, ReLU
            for c, (js, je) in enumerate(CHUNKS):
                nj = je - js
                K, M = 18 * nj, 32 * nj
                off = W2_OFF[c]
                ps = pmlp.tile([M, 1024], f32, tag="psmlp", name=f"ps2_{c}")
                for h in (0, 1):
                    hs = slice(h * 512, (h + 1) * 512)
                    nc.tensor.matmul(
                        ps[:, hs], lhsT=w_sb[0:K, off : off + M],
                        rhs=h1[c][:, hs], start=True, stop=True,
                    )
                h2[c] = work.tile([M, BATCH], f16, tag=f"h2_{c}", name=f"h2_{c}")
                epilogue(h2[c][:], ps[:], bias_sb[0:M, 6 + c : 7 + c], True)
            # L3: 32nj -> 8nj, bias only; chunk outputs stack at 32-aligned
            # partition offsets of one PSUM tile -> single whole-tile
            # epilogues produce coffT_a / coffT_b directly.
            ps3a = pmlp.tile([96, 1024], f32, tag="psmlp", name="ps3a")
            ps3b = pmlp.tile([88, 1024], f32, tag="psmlp", name="ps3b")
            for c, (js, je) in enumerate(CHUNKS):
                nj = je - js
                K, M = 32 * nj, 8 * nj
                off = W3_OFF[c]
                if c < 3:
                    dst = ps3a[32 * c : 32 * c + M, :]
                else:
                    r0 = 32 * (c - 3)
                    dst = ps3b[r0 : r0 + M, :]
                for h in (0, 1):
                    hs = slice(h * 512, (h + 1) * 512)
                    nc.tensor.matmul(
                        dst[:, hs], lhsT=w_sb[0:K, off : off + M],
                        rhs=h2[c][:, hs], start=True, stop=True,
                    )
            epilogue(coffT_a[:], ps3a[:], bias_sb[0:96, 12:13], False)
            epilogue(coffT_b[:], ps3b[:], bias_sb[0:88, 13:14], False)

        # ---- main matmul: out[128b, VC3] += coffT.T @ bfm, K = 128 + 56.
        with tc.tile_pool(name="pmain", bufs=4, space="PSUM") as pmain:
            for bt in range(NB):
                bsl = slice(bt * 128, (bt + 1) * 128)
                ostrip = outp.tile([128, VC3], f32, tag="ostrip", name=f"o_{bt}")
                pstiles = []
                for p, (t0, t1) in enumerate(NT_PAIRS):
                    ps = pmain.tile([128, 1024], f32, tag="ps", name=f"ps_{bt}_{p}")
                    pstiles.append(ps)
                # K chunk a across all pairs (stationary loaded once per
                # chunk), then chunk b accumulating on top.
                for ki, (cof, kk) in enumerate(((coffT_a, 96), (coffT_b, 88))):
                    for p, (t0, t1) in enumerate(NT_PAIRS):
                        s0 = slice(NT_BOUNDS[t0], NT_BOUNDS[t0 + 1])
                        s1 = slice(NT_BOUNDS[t1], NT_BOUNDS[t1 + 1])
                        n1 = NT_BOUNDS[t1 + 1] - NT_BOUNDS[t1]
                        ps = pstiles[p]
                        nc.tensor.matmul(
                            ps[:, 0:512], lhsT=cof[:, bsl], rhs=bfm_a[0:kk, s0] if ki == 0 else bfm_b[0:kk, s0],
                            start=ki == 0, stop=ki == 1,
                        )
                        nc.tensor.matmul(
                            ps[:, 512 : 512 + n1], lhsT=cof[:, bsl],
                            rhs=bfm_a[0:kk, s1] if ki == 0 else bfm_b[0:kk, s1],
                            start=ki == 0, stop=ki == 1,
                        )
                # evacuate with the exact 2^-13 descale, alternating ACT/DVE
                for p, (t0, t1) in enumerate(NT_PAIRS):
                    n1 = NT_BOUNDS[t1 + 1] - NT_BOUNDS[t1]
                    osl = slice(NT_BOUNDS[t0], NT_BOUNDS[t0] + 512 + n1)
                    ps = pstiles[p]
                    if ep_ctr[0] % 2 == 0:
                        nc.scalar.activation(
                            ostrip[:, osl], ps[:, 0 : 512 + n1], AF.Copy,
                            scale=DESCALE,
                        )
                    else:
                        nc.vector.tensor_scalar(
                            out=ostrip[:, osl], in0=ps[:, 0 : 512 + n1],
                            scalar1=DESCALE, scalar2=None, op0=ALU.mult,
                        )
                    ep_ctr[0] += 1
                # stores alternate the two HWDGE rings
                if bt % 2 == 0:
                    nc.sync.dma_start(out=res[bsl, :], in_=ostrip[:])
                else:
                    nc.scalar.dma_start(out=res[bsl, :], in_=ostrip[:])

    nc.finalize()
    return nc


def _pack_host(pose, basis, mask, w1, b1, w2, b2, w3, b3):
    pose_t = np.ascontiguousarray(
        pose[:, 1:].reshape(BATCH, 207).T.astype(BF16)
    )  # [207, B] rows are (j, i)

    # bfm[j*8+k, v*3+c] = basis[v, k, c] * mask[v, j] * 2^13, fp16
    prod = (
        basis[None, :, :, :] * mask.T[:, :, None, None] * BSCALE
    )  # (J, V, 8, 3) f32
    bfm = np.zeros((KTOT, VPAD * 3), BF16)
    bfm[:, : N_VERT * 3] = prod.transpose(0, 2, 1, 3).reshape(KTOT, N_VERT * 3)

    w_pack = np.zeros((128, W_COLS), BF16)
    bias_all = np.zeros((128, BIAS_COLS), np.float32)
    for (js, je), o1, o2, o3 in zip(CHUNKS, W1_OFF, W2_OFF, W3_OFF):
        for t, j in enumerate(range(js, je)):
            w_pack[t * 9 : (t + 1) * 9, o1 + t * 18 : o1 + (t + 1) * 18] = w1[j]
            w_pack[t * 18 : (t + 1) * 18, o2 + t * 32 : o2 + (t + 1) * 32] = w2[j]
            w_pack[t * 32 : (t + 1) * 32, o3 + t * 8 : o3 + (t + 1) * 8] = w3[j]
    for c, (js, je) in enumerate(CHUNKS):
        nj = je - js
        bias_all[0 : 18 * nj, c] = b1[js:je].reshape(-1)
        bias_all[0 : 32 * nj, 6 + c] = b2[js:je].reshape(-1)
        ev = np.zeros((nj, 9), np.float32)
        ev[:, [0, 4, 8]] = 1.0
        bias_all[0 : 9 * nj, 14 + c] = ev.reshape(-1)
    bias_all[0:96, 12] = b3[0:12].reshape(-1)
    bias_all[0:88, 13] = b3[12:23].reshape(-1)

    return pose_t, bfm, w_pack, bias_all


def _in_maps(pose, basis, mask, w1, b1, w2, b2, w3, b3):
    pose_t, bfm, w_pack, bias_all = _pack_host(
        np.asarray(pose, np.float32),
        np.asarray(basis, np.float32),
        np.asarray(mask, np.float32),
        np.asarray(w1, np.float32),
        np.asarray(b1, np.float32),
        np.asarray(w2, np.float32),
        np.asarray(b2, np.float32),
        np.asarray(w3, np.float32),
        np.asarray(b3, np.float32),
    )
    maps = []
    for i in range(8):
        c0 = i * VC3
        maps.append(
            {
                "pose_t": pose_t,
                "bfm_t": np.ascontiguousarray(bfm[:, c0 : c0 + VC3]),
                "w_all": w_pack,
                "bias_all": bias_all,
            }
        )
    return maps


def kernel(pose, basis, mask, w1, b1, w2, b2, w3, b3):
    from concourse.bass_utils import run_bass_kernel_spmd

    if "nc" not in _CACHED:
        _CACHED["nc"] = _build_nc()
    nc = _CACHED["nc"]

    maps = _in_maps(pose, basis, mask, w1, b1, w2, b2, w3, b3)
    r = run_bass_kernel_spmd(nc, maps, core_ids=list(range(8)))
    out = np.concatenate(
        [m["res"].reshape(BATCH, VC, 3) for m in r.results], axis=1
    )
    return np.ascontiguousarray(out[:, :N_VERT, :])


# revision 5
# speedup vs baseline: 1.3503x; 1.0546x over previous
"""BlendShapes model kernel for 8 Trainium2 NeuronCores.

Computation (reference):
    pose_repr = pose[:, 1:].reshape(B, 23, 9) - eye      # (B, J, 9)
    per-joint MLP 9 -> 18 -> 32 -> 8 (ReLU between)      # coff (B, J, 8)
    basis_full = basis[:, None] * mask[:, :, None, None]  # (V, J, 8, 3)
    res = einsum('bjk,vjkc->bvc', coff, basis_full)       # (B, V, 3)

Mapping:
  - Vertices sharded across 8 cores (V=6890 padded to 8*864=6912).
  - basis*mask (both pure inputs) is folded on the host into one fp16
    tensor bfm [184, Vc*3] with an exact 2^13 scale (fp16-subnormal guard);
    the PSUM evacuation multiplies by 2^-13 while converting to f32.
  - Each core runs the full MLP with activations transposed ([feat, batch])
    so coff^T feeds the main matmul's stationary operand directly.
  - Joints packed block-diagonally in chunks of 4 (tail 3); both 512-wide
    batch halves of a chunk go into one [M, 1024] PSUM tile -> one epilogue.
  - L3 chunk outputs land at 32-aligned partition offsets of a single PSUM
    tile, so coffT_a/[b] are produced by single whole-tile epilogues.
  - Warm-up matmuls on a zero tile keep the PE active during the input DMA
    phase so the clock ramp (PE_HAM) reaches 2.4 GHz before the MLP.
  - Main matmul: per 128-batch tile, K=184 split 128+56, N tiled in pairs
    sharing a 2-bank PSUM tile; evacuations alternate ACT/DVE; output
    stores alternate the two HWDGE rings (sync / scalar).
"""

import numpy as np
import ml_dtypes

BF16 = ml_dtypes.bfloat16

N_VERT, N_JOINT, BPJ, BATCH = 6890, 23, 8, 1024
VPAD = 6912  # 8 * 864
VC = VPAD // 8  # 864 vertices per core
VC3 = VC * 3  # 2592
KTOT = N_JOINT * BPJ  # 184
NT_BOUNDS = [0, 512, 1024, 1536, 2048, 2560, 2592]
NT_PAIRS = [(0, 1), (2, 3), (4, 5)]
NB = BATCH // 128  # 8 b-tiles

CHUNKS = [(0, 4), (4, 8), (8, 12), (12, 16), (16, 20), (20, 23)]


def _offsets(mpj):
    offs, col = [], 0
    for js, je in CHUNKS:
        offs.append(col)
        col += (je - js) * mpj
    return offs, col


W1_OFF, W1_TOT = _offsets(18)  # 414
W2_OFF, W2_TOT = _offsets(32)  # 736
W3_OFF, W3_TOT = _offsets(8)   # 184
W2_OFF = [W1_TOT + o for o in W2_OFF]
W3_OFF = [W1_TOT + W2_TOT + o for o in W3_OFF]
W_COLS = W1_TOT + W2_TOT + W3_TOT  # 1334

# bias_all columns: [0:6] L1 bias, [6:12] L2 bias, [12] L3 bias rows for
# coffT_a (joints 0-15, j-major), [13] same for coffT_b (joints 16-22),
# [14:20] eye vectors per chunk (for pose_repr = pose - eye).
BIAS_COLS = 20
BSCALE = 8192.0  # 2**13, exact in fp16 range after product
DESCALE = 1.0 / 8192.0  # exact in f32
NWARM = 16

_CACHED = {}


def _build_nc():
    import concourse.tile as tile
    from concourse import bacc, mybir
    from contextlib import ExitStack

    dt = mybir.dt
    f32, f16 = dt.float32, dt.bfloat16
    AF = mybir.ActivationFunctionType
    ALU = mybir.AluOpType

    nc = bacc.Bacc(None, target_bir_lowering=False)

    pose_t = nc.dram_tensor("pose_t", [207, BATCH], f16, kind="ExternalInput")
    bfm_t = nc.dram_tensor("bfm_t", [KTOT, VC3], f16, kind="ExternalInput")
    w_all = nc.dram_tensor("w_all", [128, W_COLS], f16, kind="ExternalInput")
    bias_all = nc.dram_tensor("bias_all", [128, BIAS_COLS], f32, kind="ExternalInput")
    res = nc.dram_tensor("res", [BATCH, VC3], f32, kind="ExternalOutput")

    with ExitStack() as ctx:
        tc = ctx.enter_context(tile.TileContext(nc))
        const = ctx.enter_context(tc.tile_pool(name="const", bufs=1))
        work = ctx.enter_context(tc.tile_pool(name="work", bufs=1))
        outp = ctx.enter_context(tc.tile_pool(name="outp", bufs=4))

        # ---- input DMAs. sync ring: bias+weights (gates LDWEIGHTS);
        # scalar ring: pose chunks (gate eye-sub/L1) then bfm (needed at main).
        bias_sb = const.tile([128, BIAS_COLS], f32, tag="bias")
        nc.sync.dma_start(out=bias_sb[:], in_=bias_all[:, :])
        w_sb = const.tile([128, W_COLS], f16, tag="w")
        nc.sync.dma_start(out=w_sb[:], in_=w_all[:, :])
        pose_c = []
        for c, (js, je) in enumerate(CHUNKS):
            K = 9 * (je - js)
            t = work.tile([K, BATCH], f16, tag=f"pose_{c}", name=f"pose_{c}")
            nc.scalar.dma_start(out=t[:], in_=pose_t[9 * js : 9 * js + K, :])
            pose_c.append(t)

        bfm_a = work.tile([96, VC3], f16, tag="bfm_a")
        bfm_b = work.tile([88, VC3], f16, tag="bfm_b")
        nc.scalar.dma_start(out=bfm_a[:], in_=bfm_t[0:96, :])
        nc.scalar.dma_start(out=bfm_b[:], in_=bfm_t[96:KTOT, :])

        # pose_repr = pose - eye (in place, fp16, DVE 2x mode)
        for c, (js, je) in enumerate(CHUNKS):
            K = 9 * (je - js)
            nc.vector.tensor_scalar(
                out=pose_c[c][:],
                in0=pose_c[c][:],
                scalar1=bias_sb[0:K, 14 + c : 15 + c],
                scalar2=None,
                op0=ALU.subtract,
            )

        coffT_a = work.tile([96, BATCH], f16, tag="coffT_a")
        coffT_b = work.tile([88, BATCH], f16, tag="coffT_b")
        h1 = {}
        h2 = {}
        ep_ctr = [0]

        def epilogue(dst, ps, bias_ap, relu):
            # Alternate ACT / DVE so consecutive epilogues overlap.
            use_act = ep_ctr[0] % 2 == 0
            ep_ctr[0] += 1
            if relu and use_act:
                nc.scalar.activation(dst, ps, AF.Relu, bias=bias_ap)
            elif relu:
                nc.vector.tensor_scalar(
                    out=dst, in0=ps, scalar1=bias_ap, scalar2=0.0,
                    op0=ALU.add, op1=ALU.max,
                )
            else:
                nc.vector.tensor_scalar(
                    out=dst, in0=ps, scalar1=bias_ap, scalar2=None, op0=ALU.add
                )

        # ---- MLP, both batch halves per chunk into one [M, 1024] PSUM tile.
        with tc.tile_pool(name="pmlp", bufs=3, space="PSUM") as pmlp:
            # L1: 9nj -> 18nj, ReLU(x + b)
            for c, (js, je) in enumerate(CHUNKS):
                nj = je - js
                K, M = 9 * nj, 18 * nj
                off = W1_OFF[c]
                ps = pmlp.tile([M, 1024], f32, tag="psmlp", name=f"ps1_{c}")
                for h in (0, 1):
                    hs = slice(h * 512, (h + 1) * 512)
                    nc.tensor.matmul(
                        ps[:, hs], lhsT=w_sb[0:K, off : off + M],
                        rhs=pose_c[c][:, hs], start=True, stop=True,
                    )
                h1[c] = work.tile([M, BATCH], f16, tag=f"h1_{c}", name=f"h1_{c}")
                epilogue(h1[c][:], ps[:], bias_sb[0:M, c : c + 1], True)
            # L2: 18nj -> 32nj, ReLU
            for c, (js, je) in enumerate(CHUNKS):
                nj = je - js
                K, M = 18 * nj, 32 * nj
                off = W2_OFF[c]
                ps = pmlp.tile([M, 1024], f32, tag="psmlp", name=f"ps2_{c}")
                for h in (0, 1):
                    hs = slice(h * 512, (h + 1) * 512)
                    nc.tensor.matmul(
                        ps[:, hs], lhsT=w_sb[0:K, off : off + M],
                        rhs=h1[c][:, hs], start=True, stop=True,
                    )
                h2[c] = work.tile([M, BATCH], f16, tag=f"h2_{c}", name=f"h2_{c}")
                epilogue(h2[c][:], ps[:], bias_sb[0:M, 6 + c : 7 + c], True)
            # L3: 32nj -> 8nj, bias only; chunk outputs stack at 32-aligned
            # partition offsets of one PSUM tile -> single whole-tile
            # epilogues produce coffT_a / coffT_b directly.
            ps3a = pmlp.tile([96, 1024], f32, tag="psmlp", name="ps3a")
            ps3b = pmlp.tile([88, 1024], f32, tag="psmlp", name="ps3b")
            for c, (js, je) in enumerate(CHUNKS):
                nj = je - js
                K, M = 32 * nj, 8 * nj
                off = W3_OFF[c]
                if c < 3:
                    dst = ps3a[32 * c : 32 * c + M, :]
                else:
                    r0 = 32 * (c - 3)
                    dst = ps3b[r0 : r0 + M, :]
                for h in (0, 1):
                    hs = slice(h * 512, (h + 1) * 512)
                    nc.tensor.matmul(
                        dst[:, hs], lhsT=w_sb[0:K, off : off + M],
                        rhs=h2[c][:, hs], start=True, stop=True,
                    )
            epilogue(coffT_a[:], ps3a[:], bias_sb[0:96, 12:13], False)
            epilogue(coffT_b[:], ps3b[:], bias_sb[0:88, 13:14], False)

        # ---- main matmul: out[128b, VC3] += coffT.T @ bfm, K = 128 + 56.
        with tc.tile_pool(name="pmain", bufs=4, space="PSUM") as pmain:
            for bt in range(NB):
                bsl = slice(bt * 128, (bt + 1) * 128)
                ostrip = outp.tile([128, VC3], f32, tag="ostrip", name=f"o_{bt}")
                pstiles = []
                for p, (t0, t1) in enumerate(NT_PAIRS):
                    ps = pmain.tile([128, 1024], f32, tag="ps", name=f"ps_{bt}_{p}")
                    pstiles.append(ps)
                # K chunk a across all pairs (stationary loaded once per
                # chunk), then chunk b accumulating on top.
                for ki, (cof, kk) in enumerate(((coffT_a, 96), (coffT_b, 88))):
                    for p, (t0, t1) in enumerate(NT_PAIRS):
                        s0 = slice(NT_BOUNDS[t0], NT_BOUNDS[t0 + 1])
                        s1 = slice(NT_BOUNDS[t1], NT_BOUNDS[t1 + 1])
                        n1 = NT_BOUNDS[t1 + 1] - NT_BOUNDS[t1]
                        ps = pstiles[p]
                        nc.tensor.matmul(
                            ps[:, 0:512], lhsT=cof[:, bsl], rhs=bfm_a[0:kk, s0] if ki == 0 else bfm_b[0:kk, s0],
                            start=ki == 0, stop=ki == 1,
                        )
                        nc.tensor.matmul(
                            ps[:, 512 : 512 + n1], lhsT=cof[:, bsl],
                            rhs=bfm_a[0:kk, s1] if ki == 0 else bfm_b[0:kk, s1],
                            start=ki == 0, stop=ki == 1,
                        )
                # evacuate with the exact 2^-13 descale, alternating ACT/DVE
                for p, (t0, t1) in enumerate(NT_PAIRS):
                    n1 = NT_BOUNDS[t1 + 1] - NT_BOUNDS[t1]
                    osl = slice(NT_BOUNDS[t0], NT_BOUNDS[t0] + 512 + n1)
                    ps = pstiles[p]
                    if ep_ctr[0] % 2 == 0:
                        nc.scalar.activation(
                            ostrip[:, osl], ps[:, 0 : 512 + n1], AF.Copy,
                            scale=DESCALE,
                        )
                    else:
                        nc.vector.tensor_scalar(
                            out=ostrip[:, osl], in0=ps[:, 0 : 512 + n1],
                            scalar1=DESCALE, scalar2=None, op0=ALU.mult,
                        )
                    ep_ctr[0] += 1
                # stores split in column halves, alternating HWDGE rings
                h0 = VC3 // 2  # 1296
                eng0 = nc.sync if bt % 2 == 0 else nc.scalar
                eng1 = nc.scalar if bt % 2 == 0 else nc.sync
                eng0.dma_start(out=res[bsl, 0:h0], in_=ostrip[:, 0:h0])
                eng1.dma_start(out=res[bsl, h0:VC3], in_=ostrip[:, h0:VC3])

    nc.finalize()
    return nc


def _pack_host(pose, basis, mask, w1, b1, w2, b2, w3, b3):
    pose_t = np.ascontiguousarray(
        pose[:, 1:].reshape(BATCH, 207).T.astype(BF16)
    )  # [207, B] rows are (j, i)

    # bfm[j*8+k, v*3+c] = basis[v, k, c] * mask[v, j] * 2^13, fp16
    prod = (
        basis[None, :, :, :] * mask.T[:, :, None, None] * BSCALE
    )  # (J, V, 8, 3) f32
    bfm = np.zeros((KTOT, VPAD * 3), BF16)
    bfm[:, : N_VERT * 3] = prod.transpose(0, 2, 1, 3).reshape(KTOT, N_VERT * 3)

    w_pack = np.zeros((128, W_COLS), BF16)
    bias_all = np.zeros((128, BIAS_COLS), np.float32)
    for (js, je), o1, o2, o3 in zip(CHUNKS, W1_OFF, W2_OFF, W3_OFF):
        for t, j in enumerate(range(js, je)):
            w_pack[t * 9 : (t + 1) * 9, o1 + t * 18 : o1 + (t + 1) * 18] = w1[j]
            w_pack[t * 18 : (t + 1) * 18, o2 + t * 32 : o2 + (t + 1) * 32] = w2[j]
            w_pack[t * 32 : (t + 1) * 32, o3 + t * 8 : o3 + (t + 1) * 8] = w3[j]
    for c, (js, je) in enumerate(CHUNKS):
        nj = je - js
        bias_all[0 : 18 * nj, c] = b1[js:je].reshape(-1)
        bias_all[0 : 32 * nj, 6 + c] = b2[js:je].reshape(-1)
        ev = np.zeros((nj, 9), np.float32)
        ev[:, [0, 4, 8]] = 1.0
        bias_all[0 : 9 * nj, 14 + c] = ev.reshape(-1)
    bias_all[0:96, 12] = b3[0:12].reshape(-1)
    bias_all[0:88, 13] = b3[12:23].reshape(-1)

    return pose_t, bfm, w_pack, bias_all


def _in_maps(pose, basis, mask, w1, b1, w2, b2, w3, b3):
    pose_t, bfm, w_pack, bias_all = _pack_host(
        np.asarray(pose, np.float32),
        np.asarray(basis, np.float32),
        np.asarray(mask, np.float32),
        np.asarray(w1, np.float32),
        np.asarray(b1, np.float32),
        np.asarray(w2, np.float32),
        np.asarray(b2, np.float32),
        np.asarray(w3, np.float32),
        np.asarray(b3, np.float32),
    )
    maps = []
    for i in range(8):
        c0 = i * VC3
        maps.append(
            {
                "pose_t": pose_t,
                "bfm_t": np.ascontiguousarray(bfm[:, c0 : c0 + VC3]),
                "w_all": w_pack,
                "bias_all": bias_all,
            }
        )
    return maps


def kernel(pose, basis, mask, w1, b1, w2, b2, w3, b3):
    from concourse.bass_utils import run_bass_kernel_spmd

    if "nc" not in _CACHED:
        _CACHED["nc"] = _build_nc()
    nc = _CACHED["nc"]

    maps = _in_maps(pose, basis, mask, w1, b1, w2, b2, w3, b3)
    r = run_bass_kernel_spmd(nc, maps, core_ids=list(range(8)))
    out = np.concatenate(
        [m["res"].reshape(BATCH, VC, 3) for m in r.results], axis=1
    )
    return np.ascontiguousarray(out[:, :N_VERT, :])


# revision 6
# speedup vs baseline: 1.4767x; 1.0936x over previous
"""BlendShapes model kernel for 8 Trainium2 NeuronCores.

Computation (reference):
    pose_repr = pose[:, 1:].reshape(B, 23, 9) - eye      # (B, J, 9)
    per-joint MLP 9 -> 18 -> 32 -> 8 (ReLU between)      # coff (B, J, 8)
    basis_full = basis[:, None] * mask[:, :, None, None]  # (V, J, 8, 3)
    res = einsum('bjk,vjkc->bvc', coff, basis_full)       # (B, V, 3)

Mapping (2-way batch x 4-way vertex shard):
  - Core i handles batch half i//4 (512 rows) and vertex quarter i%4
    (1728 of 6912 padded vertices). Halving batch per core halves the
    (otherwise replicated) MLP work; the main-matmul and store volumes
    per core are unchanged.
  - basis*mask is folded on the host into one bf16 tensor bfm with an
    exact 2^13 scale; PSUM evacuation applies 2^-13 while widening to f32.
  - MLP activations are transposed ([feat, batch]) so coff^T feeds the
    main matmul's stationary operand directly. Joints are packed
    block-diagonally in chunks of 4 (tail 3); L3 chunk outputs stack at
    32-aligned partition offsets of two PSUM tiles (bases 0/32/64), so
    coffT_a (joints 0-11, K=96) / coffT_b (joints 12-22, K=88) come from
    single whole-tile epilogues. Main K split is 96 + 88.
  - Main matmul: 8 virtual tiles (4 batch-tiles x 2 column halves of
    5184); per tile, N pairs share a 2-bank PSUM tile, K-pass-outer so
    the stationary operand is reused across the 3 pairs; evacuations
    alternate ACT/DVE; stores all go on the sync HWDGE ring (the scalar
    ring only carries input loads, keeping ACT free for epilogues).
"""

import numpy as np
import ml_dtypes

BF16 = ml_dtypes.bfloat16

N_VERT, N_JOINT, BPJ, BATCH = 6890, 23, 8, 1024
VPAD = 6912
BC = BATCH // 2  # 512 batch rows per core
VC = VPAD // 4  # 1728 vertices per core
VC3 = VC * 3  # 5184 columns per core
VH = VC3 // 2  # 2592, virtual-tile column extent
KTOT = N_JOINT * BPJ  # 184
NT_BOUNDS = [0, 512, 1024, 1536, 2048, 2560, 2592]
NT_PAIRS = [(0, 1), (2, 3), (4, 5)]
NBT = BC // 128  # 4 batch tiles per core

CHUNKS = [(0, 4), (4, 8), (8, 12), (12, 16), (16, 20), (20, 23)]


def _offsets(mpj):
    offs, col = [], 0
    for js, je in CHUNKS:
        offs.append(col)
        col += (je - js) * mpj
    return offs, col


W1_OFF, W1_TOT = _offsets(18)  # 414
W2_OFF, W2_TOT = _offsets(32)  # 736
W3_OFF, W3_TOT = _offsets(8)   # 184
W2_OFF = [W1_TOT + o for o in W2_OFF]
W3_OFF = [W1_TOT + W2_TOT + o for o in W3_OFF]
W_COLS = W1_TOT + W2_TOT + W3_TOT  # 1334

# bias_all columns: [0:6] L1 bias, [6:12] L2 bias, [12] L3 bias rows for
# coffT_a (joints 0-11, j-major), [13] same for coffT_b (joints 12-22),
# [14:20] eye vectors per chunk (for pose_repr = pose - eye).
BIAS_COLS = 20
BSCALE = 8192.0  # 2**13
DESCALE = 1.0 / 8192.0

_CACHED = {}


def _build_nc():
    import concourse.tile as tile
    from concourse import bacc, mybir
    from contextlib import ExitStack

    dt = mybir.dt
    f32, f16 = dt.float32, dt.bfloat16
    AF = mybir.ActivationFunctionType
    ALU = mybir.AluOpType

    nc = bacc.Bacc(None, target_bir_lowering=False)

    pose_t = nc.dram_tensor("pose_t", [207, BC], f16, kind="ExternalInput")
    bfm_t = nc.dram_tensor("bfm_t", [KTOT, VC3], f16, kind="ExternalInput")
    w_all = nc.dram_tensor("w_all", [128, W_COLS], f16, kind="ExternalInput")
    bias_all = nc.dram_tensor("bias_all", [128, BIAS_COLS], f32, kind="ExternalInput")
    res = nc.dram_tensor("res", [BC, VC3], f32, kind="ExternalOutput")

    with ExitStack() as ctx:
        tc = ctx.enter_context(tile.TileContext(nc))
        const = ctx.enter_context(tc.tile_pool(name="const", bufs=1))
        work = ctx.enter_context(tc.tile_pool(name="work", bufs=1))
        outp = ctx.enter_context(tc.tile_pool(name="outp", bufs=4))

        # ---- input DMAs, split across both HWDGE rings so the MLP-critical
        # pieces (bias, weights, early pose chunks) land fast and in parallel.
        bias_sb = const.tile([128, BIAS_COLS], f32, tag="bias")
        nc.sync.dma_start(out=bias_sb[:], in_=bias_all[:, :])
        w_sb = const.tile([128, W_COLS], f16, tag="w")
        nc.sync.dma_start(out=w_sb[:], in_=w_all[:, :])
        pose_c = [None] * 6
        for c in (0, 2, 4):
            K = 9 * (CHUNKS[c][1] - CHUNKS[c][0])
            r0 = 9 * CHUNKS[c][0]
            t = work.tile([K, BC], f16, tag=f"pose_{c}", name=f"pose_{c}")
            nc.scalar.dma_start(out=t[:], in_=pose_t[r0 : r0 + K, :])
            pose_c[c] = t
        for c in (1, 3, 5):
            K = 9 * (CHUNKS[c][1] - CHUNKS[c][0])
            r0 = 9 * CHUNKS[c][0]
            t = work.tile([K, BC], f16, tag=f"pose_{c}", name=f"pose_{c}")
            nc.sync.dma_start(out=t[:], in_=pose_t[r0 : r0 + K, :])
            pose_c[c] = t

        bfm_a = work.tile([96, VC3], f16, tag="bfm_a")
        bfm_b = work.tile([88, VC3], f16, tag="bfm_b")
        nc.scalar.dma_start(out=bfm_a[:], in_=bfm_t[0:96, :])
        nc.scalar.dma_start(out=bfm_b[:], in_=bfm_t[96:KTOT, :])

        # pose_repr = pose - eye (in place, DVE 2x mode)
        for c, (js, je) in enumerate(CHUNKS):
            K = 9 * (je - js)
            nc.vector.tensor_scalar(
                out=pose_c[c][:],
                in0=pose_c[c][:],
                scalar1=bias_sb[0:K, 14 + c : 15 + c],
                scalar2=None,
                op0=ALU.subtract,
            )

        coffT_a = work.tile([96, BC], f16, tag="coffT_a")
        coffT_b = work.tile([88, BC], f16, tag="coffT_b")
        h1 = {}
        h2 = {}
        ep_ctr = [0]

        def epilogue(dst, ps, bias_ap, relu):
            use_act = ep_ctr[0] % 2 == 0
            ep_ctr[0] += 1
            if relu and use_act:
                nc.scalar.activation(dst, ps, AF.Relu, bias=bias_ap)
            elif relu:
                nc.vector.tensor_scalar(
                    out=dst, in0=ps, scalar1=bias_ap, scalar2=0.0,
                    op0=ALU.add, op1=ALU.max,
                )
            else:
                nc.vector.tensor_scalar(
                    out=dst, in0=ps, scalar1=bias_ap, scalar2=None, op0=ALU.add
                )

        # ---- MLP over this core's 512 batch columns.
        with tc.tile_pool(name="pmlp", bufs=3, space="PSUM") as pmlp:
            for c, (js, je) in enumerate(CHUNKS):
                nj = je - js
                K, M = 9 * nj, 18 * nj
                off = W1_OFF[c]
                ps = pmlp.tile([M, BC], f32, tag="psmlp", name=f"ps1_{c}")
                nc.tensor.matmul(
                    ps[:], lhsT=w_sb[0:K, off : off + M], rhs=pose_c[c][:],
                    start=True, stop=True,
                )
                h1[c] = work.tile([M, BC], f16, tag=f"h1_{c}", name=f"h1_{c}")
                epilogue(h1[c][:], ps[:], bias_sb[0:M, c : c + 1], True)
            for c, (js, je) in enumerate(CHUNKS):
                nj = je - js
                K, M = 18 * nj, 32 * nj
                off = W2_OFF[c]
                ps = pmlp.tile([M, BC], f32, tag="psmlp", name=f"ps2_{c}")
                nc.tensor.matmul(
                    ps[:], lhsT=w_sb[0:K, off : off + M], rhs=h1[c][:],
                    start=True, stop=True,
                )
                h2[c] = work.tile([M, BC], f16, tag=f"h2_{c}", name=f"h2_{c}")
                epilogue(h2[c][:], ps[:], bias_sb[0:M, 6 + c : 7 + c], True)
            ps3a = pmlp.tile([96, BC], f32, tag="psmlp", name="ps3a")
            ps3b = pmlp.tile([88, BC], f32, tag="psmlp", name="ps3b")
            for c, (js, je) in enumerate(CHUNKS):
                nj = je - js
                K, M = 32 * nj, 8 * nj
                off = W3_OFF[c]
                if c < 3:
                    dst = ps3a[32 * c : 32 * c + M, :]
                else:
                    r0 = 32 * (c - 3)
                    dst = ps3b[r0 : r0 + M, :]
                nc.tensor.matmul(
                    dst[:], lhsT=w_sb[0:K, off : off + M], rhs=h2[c][:],
                    start=True, stop=True,
                )
            epilogue(coffT_a[:], ps3a[:], bias_sb[0:96, 12:13], False)
            epilogue(coffT_b[:], ps3b[:], bias_sb[0:88, 13:14], False)

        # ---- main matmul over 8 virtual tiles (bt x column half).
        with tc.tile_pool(name="pmain", bufs=4, space="PSUM") as pmain:
            vtiles = [(bt, vh) for bt in range(NBT) for vh in (0, 1)]
            for vi, (bt, vh) in enumerate(vtiles):
                bsl = slice(bt * 128, (bt + 1) * 128)
                v0 = vh * VH
                ostrip = outp.tile(
                    [128, VH], f32, tag="ostrip", name=f"o_{bt}_{vh}"
                )
                pstiles = [
                    pmain.tile([128, 1024], f32, tag="ps", name=f"ps_{vi}_{p}")
                    for p in range(len(NT_PAIRS))
                ]
                for ki, (cof, bfm, kk) in enumerate(
                    ((coffT_a, bfm_a, 96), (coffT_b, bfm_b, 88))
                ):
                    for p, (t0, t1) in enumerate(NT_PAIRS):
                        s0 = slice(v0 + NT_BOUNDS[t0], v0 + NT_BOUNDS[t0 + 1])
                        s1 = slice(v0 + NT_BOUNDS[t1], v0 + NT_BOUNDS[t1 + 1])
                        n1 = NT_BOUNDS[t1 + 1] - NT_BOUNDS[t1]
                        ps = pstiles[p]
                        nc.tensor.matmul(
                            ps[:, 0:512], lhsT=cof[:, bsl], rhs=bfm[0:kk, s0],
                            start=ki == 0, stop=ki == 1,
                        )
                        nc.tensor.matmul(
                            ps[:, 512 : 512 + n1], lhsT=cof[:, bsl],
                            rhs=bfm[0:kk, s1], start=ki == 0, stop=ki == 1,
                        )
                for p, (t0, t1) in enumerate(NT_PAIRS):
                    n1 = NT_BOUNDS[t1 + 1] - NT_BOUNDS[t1]
                    osl = slice(NT_BOUNDS[t0], NT_BOUNDS[t0] + 512 + n1)
                    ps = pstiles[p]
                    if ep_ctr[0] % 2 == 0:
                        nc.scalar.activation(
                            ostrip[:, osl], ps[:, 0 : 512 + n1], AF.Copy,
                            scale=DESCALE,
                        )
                    else:
                        nc.vector.tensor_scalar(
                            out=ostrip[:, osl], in0=ps[:, 0 : 512 + n1],
                            scalar1=DESCALE, scalar2=None, op0=ALU.mult,
                        )
                    ep_ctr[0] += 1
                # stores on the sync ring; split finer on the final tile so
                # the trailing drain is short.
                nsp = 4 if vi == len(vtiles) - 1 else 2
                step = VH // nsp
                for s in range(nsp):
                    c0 = s * step
                    nc.sync.dma_start(
                        out=res[bsl, v0 + c0 : v0 + c0 + step],
                        in_=ostrip[:, c0 : c0 + step],
                    )

    nc.finalize()
    return nc


def _pack_host(pose, basis, mask, w1, b1, w2, b2, w3, b3):
    pose_tt = np.ascontiguousarray(
        pose[:, 1:].reshape(BATCH, 207).T.astype(BF16)
    )  # [207, B] rows are (j, i)

    # bfm[j*8+k, v*3+c] = basis[v, k, c] * mask[v, j] * 2^13
    prod = (
        basis[None, :, :, :] * mask.T[:, :, None, None] * BSCALE
    )  # (J, V, 8, 3) f32
    bfm = np.zeros((KTOT, VPAD * 3), BF16)
    bfm[:, : N_VERT * 3] = prod.transpose(0, 2, 1, 3).reshape(KTOT, N_VERT * 3)

    w_pack = np.zeros((128, W_COLS), BF16)
    bias_all = np.zeros((128, BIAS_COLS), np.float32)
    for (js, je), o1, o2, o3 in zip(CHUNKS, W1_OFF, W2_OFF, W3_OFF):
        for t, j in enumerate(range(js, je)):
            w_pack[t * 9 : (t + 1) * 9, o1 + t * 18 : o1 + (t + 1) * 18] = w1[j]
            w_pack[t * 18 : (t + 1) * 18, o2 + t * 32 : o2 + (t + 1) * 32] = w2[j]
            w_pack[t * 32 : (t + 1) * 32, o3 + t * 8 : o3 + (t + 1) * 8] = w3[j]
    for c, (js, je) in enumerate(CHUNKS):
        nj = je - js
        bias_all[0 : 18 * nj, c] = b1[js:je].reshape(-1)
        bias_all[0 : 32 * nj, 6 + c] = b2[js:je].reshape(-1)
        ev = np.zeros((nj, 9), np.float32)
        ev[:, [0, 4, 8]] = 1.0
        bias_all[0 : 9 * nj, 14 + c] = ev.reshape(-1)
    bias_all[0:96, 12] = b3[0:12].reshape(-1)
    bias_all[0:88, 13] = b3[12:23].reshape(-1)

    return pose_tt, bfm, w_pack, bias_all


def _in_maps(pose, basis, mask, w1, b1, w2, b2, w3, b3):
    pose_tt, bfm, w_pack, bias_all = _pack_host(
        np.asarray(pose, np.float32),
        np.asarray(basis, np.float32),
        np.asarray(mask, np.float32),
        np.asarray(w1, np.float32),
        np.asarray(b1, np.float32),
        np.asarray(w2, np.float32),
        np.asarray(b2, np.float32),
        np.asarray(w3, np.float32),
        np.asarray(b3, np.float32),
    )
    maps = []
    for i in range(8):
        bh, vq = i // 4, i % 4
        maps.append(
            {
                "pose_t": np.ascontiguousarray(
                    pose_tt[:, bh * BC : (bh + 1) * BC]
                ),
                "bfm_t": np.ascontiguousarray(
                    bfm[:, vq * VC3 : (vq + 1) * VC3]
                ),
                "w_all": w_pack,
                "bias_all": bias_all,
            }
        )
    return maps


def kernel(pose, basis, mask, w1, b1, w2, b2, w3, b3):
    from concourse.bass_utils import run_bass_kernel_spmd

    if "nc" not in _CACHED:
        _CACHED["nc"] = _build_nc()
    nc = _CACHED["nc"]

    maps = _in_maps(pose, basis, mask, w1, b1, w2, b2, w3, b3)
    r = run_bass_kernel_spmd(nc, maps, core_ids=list(range(8)))
    full = np.empty((BATCH, VPAD * 3), np.float32)
    for i in range(8):
        bh, vq = i // 4, i % 4
        full[bh * BC : (bh + 1) * BC, vq * VC3 : (vq + 1) * VC3] = r.results[i][
            "res"
        ]
    out = full.reshape(BATCH, VPAD, 3)
    return np.ascontiguousarray(out[:, :N_VERT, :])
